# revision 1
# baseline (speedup 1.0000x reference)
"""Trainium2 Bass kernel for nn_MultiHeadDecoder (sparse neighbour compat + MLP + softmax).

Strategy (data-parallel over batch, 8 batches per core):
 - Host: decompose the `rec` permutation into cycles and lay nodes out in tour
   order (with per-cycle pad columns) so predecessor / succ^2 lookups become
   free-dim column shifts on-chip.  The per-core shard is shipped pre-gathered
   and feature-major: hemt[b] = h_em[b][order].T  ([128, PEXT]).
 - Algebra folding (host, float64): the reference's per-head Q/K projections of
   h = h_em @ Wn.T + g-proj collapse into one bilinear form per head:
       compat[pos p] = (A_h[:,p-1]+c_h).F[:,p] + (A_h[:,p]+c_h).E[:,p+2]  (+s)
   where A_h = Mt_h^T E, Mt_h = Wn^T Wq_h Wk_h^T Wn, F = E - shift2(E), and the
   per-batch scalars c_h (from the graph-max projection) ride the DVE op while
   s folds into the first MLP bias.  This halves matmul work vs explicit Q/K.
 - Device: one 128x128 matmul per head (PSUM), fused (A+c)*F products on DVE
   straight out of PSUM (scalar_tensor_tensor), per-position dot reduction as a
   TensorE column-sum matmul (lhsT=ones) writing compat feature-major
   [4 heads, positions], GPSIMD ap_gather to join pickup/delivery tour
   positions into node order, then the 12->32->32->1 MLP + tanh + softmax.
"""
import os
import sys
from contextlib import ExitStack

import numpy as np

for _p in ("/opt/trn_rl_repo", "/root/.axon_site/_ro/trn_rl_repo"):
    if os.path.isdir(_p) and _p not in sys.path:
        sys.path.insert(0, _p)

import concourse.bacc as bacc
import concourse.bass as bass
import concourse.mybir as mybir
import concourse.tile as tile
from concourse.bass_utils import run_bass_kernel_spmd
from concourse.library_config import mlp as _mlp_lib

F32 = mybir.dt.float32
BF16 = mybir.dt.bfloat16
BS, GS, D, NH = 64, 2001, 128, 4
N = GS // 2                 # 1000
NCORES = 8
BPC = BS // NCORES          # 8 batches per core
PEXT = 2048                 # extended tour positions (3 pads/cycle; grown if needed)
NIDX = 1008                 # padded gather count (>= N, %16 == 0)
IDXW = NIDX // 16           # 63
MLP_CHUNKS = [(0, 512), (512, 488)]

_CACHE = {}


def _chunks():
    out = []
    c0 = 0
    while c0 < PEXT:
        out.append((c0, min(512, PEXT - c0)))
        c0 += 512
    return out


def _build_nc():
    CHUNKS = _chunks()
    nc = bacc.Bacc(None, target_bir_lowering=False, debug=False)
    hemt_d = nc.dram_tensor("hemt", [BPC, 128, PEXT], F32, kind="ExternalInput")
    sig_d = nc.dram_tensor("sig", [BPC, 4, N], F32, kind="ExternalInput")
    pdidx_d = nc.dram_tensor("pdidx", [BPC, 16, 2 * IDXW], mybir.dt.int16, kind="ExternalInput")
    mt_d = nc.dram_tensor("mt", [128, NH * 128], F32, kind="ExternalInput")
    cvec_d = nc.dram_tensor("cvec", [BPC, 128, NH], F32, kind="ExternalInput")
    w1p_d = nc.dram_tensor("w1p", [4, 32], F32, kind="ExternalInput")
    w1d_d = nc.dram_tensor("w1d", [4, 32], F32, kind="ExternalInput")
    w1s_d = nc.dram_tensor("w1s", [4, 32], F32, kind="ExternalInput")
    b1e_d = nc.dram_tensor("b1e", [32, BPC], F32, kind="ExternalInput")
    w2t_d = nc.dram_tensor("w2t", [32, 32], F32, kind="ExternalInput")
    b2_d = nc.dram_tensor("b2", [32, 1], F32, kind="ExternalInput")
    w3t_d = nc.dram_tensor("w3t", [32, 1], F32, kind="ExternalInput")
    b3_d = nc.dram_tensor("b3", [1, 1], F32, kind="ExternalInput")
    hmask_d = nc.dram_tensor("hmask", [128, NH * NH], F32, kind="ExternalInput")
    out_d = nc.dram_tensor("out", [BPC, N], F32, kind="ExternalOutput")

    with tile.TileContext(nc) as tc:
        with ExitStack() as ctx:
            const = ctx.enter_context(tc.tile_pool(name="const", bufs=1))
            inpool = ctx.enter_context(tc.tile_pool(name="inpool", bufs=2))
            epool = ctx.enter_context(
                tc.tile_pool(name="epool", bufs=2 if PEXT <= 2048 else 1))
            ppool = ctx.enter_context(tc.tile_pool(name="ppool", bufs=1))
            spool = ctx.enter_context(tc.tile_pool(name="spool", bufs=2))
            a_ps = ctx.enter_context(tc.tile_pool(name="a_ps", bufs=2, space="PSUM"))
            cs_ps = ctx.enter_context(tc.tile_pool(name="cs_ps", bufs=2, space="PSUM"))
            mlp_ps = ctx.enter_context(tc.tile_pool(name="mlp_ps", bufs=2, space="PSUM"))

            nc.gpsimd.load_library(_mlp_lib)

            hmask_f32 = const.tile([128, NH * NH], F32)
            nc.sync.dma_start(out=hmask_f32[:], in_=hmask_d[:])
            hmask_sb = const.tile([128, NH * NH], BF16)
            nc.vector.tensor_copy(out=hmask_sb[:], in_=hmask_f32[:])
            mt_sb = const.tile([128, NH * 128], F32)
            nc.sync.dma_start(out=mt_sb[:], in_=mt_d[:])
            w1p_sb = const.tile([4, 32], F32)
            nc.sync.dma_start(out=w1p_sb[:], in_=w1p_d[:])
            w1d_sb = const.tile([4, 32], F32)
            nc.sync.dma_start(out=w1d_sb[:], in_=w1d_d[:])
            w1s_sb = const.tile([4, 32], F32)
            nc.sync.dma_start(out=w1s_sb[:], in_=w1s_d[:])
            b1e_sb = const.tile([32, BPC], F32)
            nc.sync.dma_start(out=b1e_sb[:], in_=b1e_d[:])
            w2t_sb = const.tile([32, 32], F32)
            nc.sync.dma_start(out=w2t_sb[:], in_=w2t_d[:])
            b2_sb = const.tile([32, 1], F32)
            nc.sync.dma_start(out=b2_sb[:], in_=b2_d[:])
            w3t_sb = const.tile([32, 1], F32)
            nc.sync.dma_start(out=w3t_sb[:], in_=w3t_d[:])
            b3_sb = const.tile([1, 1], F32)
            nc.sync.dma_start(out=b3_sb[:], in_=b3_d[:])
            compat2 = []
            for i in range(2):
                t = const.tile([16, PEXT], F32, name=f"compat{i}")
                nc.gpsimd.memset(t[:], 0.0)
                compat2.append(t)

            for b in range(BPC):
                pdidx_sb = inpool.tile([16, 2 * IDXW], mybir.dt.int16)
                nc.sync.dma_start(out=pdidx_sb[:], in_=pdidx_d[b])
                cv_sb = inpool.tile([128, NH], F32)
                nc.sync.dma_start(out=cv_sb[:], in_=cvec_d[b])
                sig_sb = inpool.tile([4, N], F32)
                nc.sync.dma_start(out=sig_sb[:], in_=sig_d[b])

                e_fm = epool.tile([128, PEXT], F32)
                nc.sync.dma_start(out=e_fm[:], in_=hemt_d[b])

                # bf16 copies for the 2x-mode product stage
                e_bf = epool.tile([128, PEXT], BF16)
                nc.vector.tensor_copy(out=e_bf[:], in_=e_fm[:])
                # fm1[:, j] = F[:, j+1] = E[:, j+1] - E[:, j+3]  (stored shifted
                # by -1 so every product op is even-aligned for 2x mode)
                fm1 = epool.tile([128, PEXT], BF16)
                nc.vector.tensor_sub(fm1[:, 0:PEXT - 3], e_fm[:, 1:PEXT - 2], e_fm[:, 3:PEXT])
                nc.vector.tensor_copy(out=fm1[:, PEXT - 3:PEXT], in_=e_fm[:, PEXT - 3:PEXT])

                # A'_h = Mt_h^T E + c_h, drained PSUM->SBUF as bf16 (ACT h<3, DVE h=3)
                a_bf = epool.tile([128, NH, PEXT], BF16)
                for h in range(NH):
                    cv = cv_sb[:, h:h + 1]
                    for pair0 in range(0, PEXT, 1024):
                        pw = min(1024, PEXT - pair0)
                        ap = a_ps.tile([128, 1024], F32, space="PSUM", tag="a")
                        for (c0, w) in [(c, min(512, pw - (c - pair0)))
                                        for c in range(pair0, pair0 + pw, 512)]:
                            nc.tensor.matmul(out=ap[:, c0 - pair0:c0 - pair0 + w],
                                             lhsT=mt_sb[:, h * 128:(h + 1) * 128],
                                             rhs=e_fm[:, c0:c0 + w], start=True, stop=True)
                        if h < 3:
                            nc.scalar.add(out=a_bf[:, h, pair0:pair0 + pw],
                                          in_=ap[:, :pw], add=cv)
                        else:
                            nc.vector.tensor_scalar_add(a_bf[:, h, pair0:pair0 + pw],
                                                        ap[:, :pw], cv)

                # products (bf16, 2x): p_sb[:, 2h, j] = A'_h[:, j] * F[:, j+1]
                #                      p_sb[:, 2h+1, j] = A'_h[:, j] * E[:, j+2]
                p_sb = ppool.tile([128, 2 * NH, PEXT], BF16)
                for h in range(NH):
                    nc.vector.tensor_mul(p_sb[:, 2 * h, :], a_bf[:, h, :], fm1[:])
                    nc.vector.tensor_mul(p_sb[:, 2 * h + 1, 0:PEXT - 2],
                                         a_bf[:, h, 0:PEXT - 2], e_bf[:, 2:PEXT])
                    nc.vector.tensor_mul(p_sb[:, 2 * h + 1, PEXT - 2:PEXT],
                                         a_bf[:, h, PEXT - 2:PEXT], e_bf[:, PEXT - 2:PEXT])

                # compat[h, pos p] = colsum(P2_h)[p] + colsum(P1_h)[p-1]
                compat_sb = compat2[b % 2]
                for (c0, w) in CHUNKS:
                    cs = cs_ps.tile([4, 512], F32, space="PSUM", tag="cs")
                    for h in range(NH):
                        mk = hmask_sb[:, h * NH:(h + 1) * NH]
                        if h < NH - 1:
                            nc.tensor.matmul(out=cs[:, :w], lhsT=mk,
                                             rhs=p_sb[:, 2 * h + 1, c0:c0 + w],
                                             start=(h == 0), stop=False)
                            t1 = (cs[:, 1:w], p_sb[:, 2 * h, 0:w - 1]) if c0 == 0 else \
                                 (cs[:, :w], p_sb[:, 2 * h, c0 - 1:c0 - 1 + w])
                            nc.tensor.matmul(out=t1[0], lhsT=mk, rhs=t1[1],
                                             start=False, stop=False,
                                             skip_group_check=True)
                        else:
                            t1 = (cs[:, 1:w], p_sb[:, 2 * h, 0:w - 1]) if c0 == 0 else \
                                 (cs[:, :w], p_sb[:, 2 * h, c0 - 1:c0 - 1 + w])
                            nc.tensor.matmul(out=t1[0], lhsT=mk, rhs=t1[1],
                                             start=False, stop=False,
                                             skip_group_check=True)
                            nc.tensor.matmul(out=cs[:, :w], lhsT=mk,
                                             rhs=p_sb[:, 2 * h + 1, c0:c0 + w],
                                             start=False, stop=True)
                    nc.scalar.copy(out=compat_sb[0:4, c0:c0 + w], in_=cs[:, :w])

                pd_g = spool.tile([16, 2 * NIDX], F32)
                nc.gpsimd.ap_gather(pd_g[:], compat_sb[:], pdidx_sb[:],
                                    channels=16, num_elems=PEXT, d=1, num_idxs=2 * NIDX)

                x1_sb = spool.tile([32, N], F32)
                x2_sb = spool.tile([32, N], F32)
                tab_sb = spool.tile([1, N], F32)
                for (c0, w) in MLP_CHUNKS:
                    x1p = mlp_ps.tile([32, 512], F32, space="PSUM", tag="m")
                    nc.tensor.matmul(out=x1p[:, :w], lhsT=w1p_sb[:], rhs=pd_g[0:4, c0:c0 + w],
                                     start=True, stop=False)
                    nc.tensor.matmul(out=x1p[:, :w], lhsT=w1d_sb[:], rhs=pd_g[0:4, NIDX + c0:NIDX + c0 + w],
                                     start=False, stop=False)
                    nc.tensor.matmul(out=x1p[:, :w], lhsT=w1s_sb[:], rhs=sig_sb[:, c0:c0 + w],
                                     start=False, stop=True)
                    nc.scalar.activation(out=x1_sb[:, c0:c0 + w], in_=x1p[:, :w],
                                         func=mybir.ActivationFunctionType.Relu,
                                         bias=b1e_sb[:, b:b + 1], scale=1.0)
                for (c0, w) in MLP_CHUNKS:
                    x2p = mlp_ps.tile([32, 512], F32, space="PSUM", tag="m")
                    nc.tensor.matmul(out=x2p[:, :w], lhsT=w2t_sb[:], rhs=x1_sb[:, c0:c0 + w],
                                     start=True, stop=True)
                    nc.scalar.activation(out=x2_sb[:, c0:c0 + w], in_=x2p[:, :w],
                                         func=mybir.ActivationFunctionType.Relu,
                                         bias=b2_sb[:], scale=1.0)
                for (c0, w) in MLP_CHUNKS:
                    tp3 = mlp_ps.tile([1, 512], F32, space="PSUM", tag="m")
                    nc.tensor.matmul(out=tp3[:, :w], lhsT=w3t_sb[:], rhs=x2_sb[:, c0:c0 + w],
                                     start=True, stop=True)
                    nc.scalar.activation(out=tab_sb[:, c0:c0 + w], in_=tp3[:, :w],
                                         func=mybir.ActivationFunctionType.Tanh,
                                         bias=b3_sb[:], scale=1.0)

                # softmax over 6*tanh; values bounded in [-6, 6] so no max-shift needed
                ex_sb = spool.tile([1, N], F32)
                ssum = spool.tile([1, 1], F32)
                nc.scalar.activation(out=ex_sb[:], in_=tab_sb[:],
                                     func=mybir.ActivationFunctionType.Exp,
                                     bias=0.0, scale=6.0, accum_out=ssum[:])
                rcp = spool.tile([1, 1], F32)
                nc.vector.reciprocal(rcp[:], ssum[:])
                probs = spool.tile([1, N], F32)
                nc.vector.tensor_scalar_mul(probs[:], ex_sb[:], rcp[:])
                nc.sync.dma_start(out=out_d[b:b + 1, :], in_=probs[:])
    nc.compile()
    return nc


def _decompose(perm):
    visited = np.zeros(GS, bool)
    order = []
    real = []
    for start in range(GS):
        if visited[start]:
            continue
        cyc = [start]
        visited[start] = True
        nxt = int(perm[start])
        while nxt != start:
            cyc.append(nxt)
            visited[nxt] = True
            nxt = int(perm[nxt])
        L = len(cyc)
        order.extend([cyc[-1]] + cyc + [cyc[0 % L], cyc[1 % L]])
        real.extend([False] + [True] * L + [False] * 2)
    assert len(order) <= PEXT, f"too many cycles: ext len {len(order)}"
    pad = PEXT - len(order)
    order.extend([0] * pad)
    real.extend([False] * pad)
    return np.asarray(order, np.int64), np.asarray(real, bool)


def _ext_len(perm):
    visited = np.zeros(GS, bool)
    ncyc = 0
    for start in range(GS):
        if not visited[start]:
            ncyc += 1
            visited[start] = True
            nxt = int(perm[start])
            while nxt != start:
                visited[nxt] = True
                nxt = int(perm[nxt])
    return GS + 3 * ncyc


def _idx_tile(ppos, dpos):
    idx = np.zeros(2 * NIDX, np.int16)
    idx[:ppos.shape[0]] = ppos
    idx[NIDX:NIDX + dpos.shape[0]] = dpos
    return idx.reshape(2 * IDXW, 16).T.copy()


def _host_prep(inputs):
    h_em = np.asarray(inputs["h_em"], np.float32)
    rec = np.asarray(inputs["rec"], np.int64)
    sig = np.ascontiguousarray(np.asarray(inputs["selection_sig"], np.float32))
    Wn = np.asarray(inputs["W_node"], np.float64)
    Wg = np.asarray(inputs["W_graph"], np.float64)
    WQ = np.asarray(inputs["W_Q"], np.float64)
    WK = np.asarray(inputs["W_K"], np.float64)
    w1 = np.asarray(inputs["agg_w1"], np.float64)
    b1 = np.asarray(inputs["agg_b1"], np.float64)
    w2 = np.asarray(inputs["agg_w2"], np.float32)
    b2 = np.asarray(inputs["agg_b2"], np.float32)
    w3 = np.asarray(inputs["agg_w3"], np.float32)
    b3 = np.asarray(inputs["agg_b3"], np.float32)

    Mt = np.zeros((NH, D, D), np.float64)
    C = np.zeros((NH, D, D), np.float64)
    S = np.zeros((NH, D, D), np.float64)
    for h in range(NH):
        M = WQ[h] @ WK[h].T
        Mt[h] = Wn.T @ M @ Wn
        C[h] = Wn.T @ (M + M.T) @ Wg
        S[h] = Wg.T @ M @ Wg
    mt = np.concatenate([Mt[h].astype(np.float32) for h in range(NH)], axis=1)

    g = h_em.max(axis=1).astype(np.float64)                      # (BS, D)
    cvec = np.einsum("hdf,bf->bdh", C, g).astype(np.float32)     # (BS, D, NH)
    svec = np.einsum("bd,hdf,bf->bh", g, S, g)                   # (BS, NH)
    b1_eff = (b1[None, :] + svec @ (w1[:, 0:4] + w1[:, 4:8]).T).astype(np.float32)

    w1f = w1.astype(np.float32)
    hmask = np.zeros((128, NH * NH), np.float32)
    for h in range(NH):
        hmask[:, h * NH + h] = 1.0
    shared = {
        "mt": mt,
        "hmask": hmask,
        "w1p": np.ascontiguousarray(w1f[:, 0:4].T),
        "w1d": np.ascontiguousarray(w1f[:, 4:8].T),
        "w1s": np.ascontiguousarray(w1f[:, 8:12].T),
        "w2t": np.ascontiguousarray(w2.T),
        "b2": b2.reshape(32, 1),
        "w3t": np.ascontiguousarray(w3.T),
        "b3": b3.reshape(1, 1),
    }

    in_maps = []
    for core in range(NCORES):
        b0 = core * BPC
        hemt = np.empty((BPC, 128, PEXT), np.float32)
        pdidx = np.empty((BPC, 16, 2 * IDXW), np.int16)
        for bl in range(BPC):
            order, real = _decompose(rec[b0 + bl])
            hemt[bl] = h_em[b0 + bl][order].T
            pon = np.zeros(GS, np.int64)
            pon[order[real]] = np.nonzero(real)[0]
            pdidx[bl] = _idx_tile(pon[1:N + 1], pon[N + 1:2 * N + 1])
        m = {
            "hemt": hemt,
            "sig": sig[b0:b0 + BPC],
            "pdidx": pdidx,
            "cvec": cvec[b0:b0 + BPC],
            "b1e": np.ascontiguousarray(b1_eff[b0:b0 + BPC].T),
        }
        m.update(shared)
        in_maps.append(m)
    return in_maps


def kernel(**inputs) -> np.ndarray:
    global PEXT
    rec = np.asarray(inputs["rec"], np.int64)
    need = max(_ext_len(rec[b]) for b in range(rec.shape[0]))
    want = max(2048, -(-need // 512) * 512)
    if want != PEXT or "nc" not in _CACHE:
        PEXT = want
        _CACHE["nc"] = _build_nc()
    nc = _CACHE["nc"]
    in_maps = _host_prep(inputs)
    res = run_bass_kernel_spmd(nc, in_maps, list(range(NCORES)))
    return np.concatenate([res.results[i]["out"] for i in range(NCORES)], axis=0)



# revision 2
# speedup vs baseline: 1.7615x; 1.7615x over previous
"""Trainium2 Bass kernel for nn_MultiHeadDecoder (sparse neighbour compat + MLP + softmax).

Strategy (data-parallel over batch, 8 batches per core):
 - Host: decompose the `rec` permutation into cycles and lay nodes out in tour
   order (with per-cycle pad columns) so predecessor / succ^2 lookups become
   free-dim column shifts on-chip.  Ship per-batch:
     hemt  = E = h_em[order].T           fp16 [128, PEXT]
     fmt   = F' where F'[j] = E[j+1]-E[j+3]  fp16 [128, PEXT]
   plus gather indices for the pickup/delivery pairing (per-16-partition-group
   index lists for gpsimd indirect_copy).
 - Algebra folding (host, float64): per-head Q/K projections collapse to one
   bilinear form per head:
     compat[p] = (A_h[:,p-1]+c_h).F[:,p] + (A_h[:,p]+c_h).E[:,p+2]  (+s)
   A_h = Mt_h^T E;  c_h rides the PSUM drain as a per-partition bias; s folds
   into the first MLP bias.
 - Device per batch: 4 fp16 128x128 matmuls (A, PSUM f32), drains with fused
   +c on ACT/DVE to fp16, 8 fp16 elementwise products (DVE 2x / Pool),
   column-sum matmuls with a head-duplicating mask writing compat to
   partitions {0-3, 16-19}, one gpsimd indirect_copy gathering pickup rows
   (group 0) and delivery rows (group 1) in node order, a row-move DMA +
   sig DMA to build a single [12, NIDX] MLP input, then the 12->32->32->1
   MLP (fp16 weights, single-stream x1) + tanh + softmax.
"""
import os
import sys
from contextlib import ExitStack

import numpy as np

for _p in ("/opt/trn_rl_repo", "/root/.axon_site/_ro/trn_rl_repo"):
    if os.path.isdir(_p) and _p not in sys.path:
        sys.path.insert(0, _p)

import concourse.bacc as bacc
import concourse.bass as bass
import concourse.mybir as mybir
import concourse.tile as tile
from concourse.bass_utils import run_bass_kernel_spmd
from concourse.library_config import mlp as _mlp_lib

F32 = mybir.dt.float32
F16 = mybir.dt.float16
BF16 = mybir.dt.bfloat16
BS, GS, D, NH = 64, 2001, 128, 4
N = GS // 2                 # 1000
NCORES = 8
BPC = BS // NCORES          # 8 batches per core
PEXT = 2048                 # extended tour positions (3 pads/cycle; grown if needed)
NIDX = 1008                 # padded gather count (>= N, %16 == 0)
IDXW = NIDX // 16           # 63
MLP_CHUNKS = [(0, 504), (504, 504)]

# engine assignment tuning knobs
N_DRAIN_DVE = 3             # of the 8 A-drain ops, how many go to DVE (rest ACT)
N_PROD_POOL = 1             # of the 8 product ops, how many go to Pool (rest DVE)

_CACHE = {}


def _chunks():
    out = []
    c0 = 0
    while c0 < PEXT:
        out.append((c0, min(512, PEXT - c0)))
        c0 += 512
    return out


def _build_nc():
    CHUNKS = _chunks()
    nc = bacc.Bacc(None, target_bir_lowering=False, debug=False)
    hemt_d = nc.dram_tensor("hemt", [BPC, 128, PEXT], F16, kind="ExternalInput")
    fmt_d = nc.dram_tensor("fmt", [BPC, 128, PEXT], F16, kind="ExternalInput")
    sig_d = nc.dram_tensor("sig", [BPC, 4, N], F32, kind="ExternalInput")
    pdidx_d = nc.dram_tensor("pdidx", [BPC, 128, IDXW], mybir.dt.uint16, kind="ExternalInput")
    mt_d = nc.dram_tensor("mt", [128, NH * 128], F16, kind="ExternalInput")
    cvec_d = nc.dram_tensor("cvec", [BPC, 128, NH], F32, kind="ExternalInput")
    w1t_d = nc.dram_tensor("w1t", [12, 32], BF16, kind="ExternalInput")
    b1e_d = nc.dram_tensor("b1e", [32, BPC], F32, kind="ExternalInput")
    w2t_d = nc.dram_tensor("w2t", [32, 32], F16, kind="ExternalInput")
    b2_d = nc.dram_tensor("b2", [32, 1], F32, kind="ExternalInput")
    w3t_d = nc.dram_tensor("w3t", [32, 1], F16, kind="ExternalInput")
    b3_d = nc.dram_tensor("b3", [1, 1], F32, kind="ExternalInput")
    hmask_d = nc.dram_tensor("hmask", [128, NH * 20], F16, kind="ExternalInput")
    out_d = nc.dram_tensor("out", [BPC, N], F32, kind="ExternalOutput")

    with tile.TileContext(nc) as tc:
        with ExitStack() as ctx:
            const = ctx.enter_context(tc.tile_pool(name="const", bufs=1))
            inpool = ctx.enter_context(tc.tile_pool(name="inpool", bufs=2))
            epool = ctx.enter_context(
                tc.tile_pool(name="epool", bufs=2 if PEXT <= 2048 else 1))
            ppool = ctx.enter_context(tc.tile_pool(name="ppool", bufs=2))
            spool = ctx.enter_context(tc.tile_pool(name="spool", bufs=2))
            a_ps = ctx.enter_context(tc.tile_pool(name="a_ps", bufs=2, space="PSUM"))
            cs_ps = ctx.enter_context(tc.tile_pool(name="cs_ps", bufs=2, space="PSUM"))
            mlp_ps = ctx.enter_context(tc.tile_pool(name="mlp_ps", bufs=2, space="PSUM"))

            nc.gpsimd.load_library(_mlp_lib)

            hmask_sb = const.tile([128, NH * 20], F16)
            nc.sync.dma_start(out=hmask_sb[:], in_=hmask_d[:])
            mt_sb = const.tile([128, NH * 128], F16)
            nc.sync.dma_start(out=mt_sb[:], in_=mt_d[:])
            w1t_sb = const.tile([12, 32], BF16)
            nc.sync.dma_start(out=w1t_sb[:], in_=w1t_d[:])
            b1e_sb = const.tile([32, BPC], F32)
            nc.sync.dma_start(out=b1e_sb[:], in_=b1e_d[:])
            w2t_sb = const.tile([32, 32], F16)
            nc.sync.dma_start(out=w2t_sb[:], in_=w2t_d[:])
            b2_sb = const.tile([32, 1], F32)
            nc.sync.dma_start(out=b2_sb[:], in_=b2_d[:])
            w3t_sb = const.tile([32, 1], F16)
            nc.sync.dma_start(out=w3t_sb[:], in_=w3t_d[:])
            b3_sb = const.tile([1, 1], F32)
            nc.sync.dma_start(out=b3_sb[:], in_=b3_d[:])
            # compat gather sources: [128, PEXT] f32, rows 0-3 & 16-19 live
            compat2 = []
            for i in range(2):
                t = const.tile([128, PEXT], F32, name=f"compat{i}")
                nc.gpsimd.memset(t[:], 0.0)
                compat2.append(t)

            for b in range(BPC):
                pdidx_sb = inpool.tile([128, IDXW], mybir.dt.uint16)
                nc.sync.dma_start(out=pdidx_sb[:], in_=pdidx_d[b])
                cv_sb = inpool.tile([128, NH], F32)
                nc.sync.dma_start(out=cv_sb[:], in_=cvec_d[b])

                e_sb = epool.tile([128, PEXT], F16)
                nc.sync.dma_start(out=e_sb[:], in_=hemt_d[b])
                f_sb = epool.tile([128, PEXT], F16)
                nc.sync.dma_start(out=f_sb[:], in_=fmt_d[b])

                # A'_h = Mt_h^T E + c_h, drained PSUM->SBUF as fp16 (ACT/DVE)
                a_bf = epool.tile([128, NH, PEXT], F16)
                drain_i = 0
                for h in range(NH):
                    cv = cv_sb[:, h:h + 1]
                    for pair0 in range(0, PEXT, 1024):
                        pw = min(1024, PEXT - pair0)
                        ap = a_ps.tile([128, 1024], F32, space="PSUM", tag="a")
                        for (c0, w) in [(c, min(512, pw - (c - pair0)))
                                        for c in range(pair0, pair0 + pw, 512)]:
                            nc.tensor.matmul(out=ap[:, c0 - pair0:c0 - pair0 + w],
                                             lhsT=mt_sb[:, h * 128:(h + 1) * 128],
                                             rhs=e_sb[:, c0:c0 + w], start=True, stop=True)
                        if drain_i % 8 < N_DRAIN_DVE:
                            nc.vector.tensor_scalar_add(a_bf[:, h, pair0:pair0 + pw],
                                                        ap[:, :pw], cv)
                        else:
                            nc.scalar.add(out=a_bf[:, h, pair0:pair0 + pw],
                                          in_=ap[:, :pw], add=cv)
                        drain_i += 1

                # products (fp16, 2x): p_sb[:, 2h, j] = A'_h[:, j] * F'[:, j]
                #   (F'[j] = F[j+1], stored shifted for even alignment)
                #                      p_sb[:, 2h+1, j] = A'_h[:, j] * E[:, j+2]
                p_sb = ppool.tile([128, 2 * NH, PEXT], F16)
                prod_i = 0
                for h in range(NH):
                    eng1 = nc.gpsimd if prod_i % 8 < N_PROD_POOL else nc.vector
                    eng1.tensor_mul(p_sb[:, 2 * h, :], a_bf[:, h, :], f_sb[:])
                    prod_i += 1
                    eng2 = nc.gpsimd if prod_i % 8 < N_PROD_POOL else nc.vector
                    eng2.tensor_mul(p_sb[:, 2 * h + 1, 0:PEXT - 2],
                                    a_bf[:, h, 0:PEXT - 2], e_sb[:, 2:PEXT])
                    eng2.tensor_mul(p_sb[:, 2 * h + 1, PEXT - 2:PEXT],
                                    a_bf[:, h, PEXT - 2:PEXT], e_sb[:, PEXT - 2:PEXT])
                    prod_i += 1

                # compat[h, pos p] = colsum(P2_h)[p] + colsum(P1_h)[p-1],
                # written to partitions h and 16+h (mask20 duplicates heads)
                compat_sb = compat2[b % 2]
                for (c0, w) in CHUNKS:
                    cs = cs_ps.tile([20, 512], F32, space="PSUM", tag="cs")
                    for h in range(NH):
                        mk = hmask_sb[:, h * 20:(h + 1) * 20]
                        if h < NH - 1:
                            nc.tensor.matmul(out=cs[:, :w], lhsT=mk,
                                             rhs=p_sb[:, 2 * h + 1, c0:c0 + w],
                                             start=(h == 0), stop=False)
                            t1 = (cs[:, 1:w], p_sb[:, 2 * h, 0:w - 1]) if c0 == 0 else \
                                 (cs[:, :w], p_sb[:, 2 * h, c0 - 1:c0 - 1 + w])
                            nc.tensor.matmul(out=t1[0], lhsT=mk, rhs=t1[1],
                                             start=False, stop=False,
                                             skip_group_check=True)
                        else:
                            t1 = (cs[:, 1:w], p_sb[:, 2 * h, 0:w - 1]) if c0 == 0 else \
                                 (cs[:, :w], p_sb[:, 2 * h, c0 - 1:c0 - 1 + w])
                            nc.tensor.matmul(out=t1[0], lhsT=mk, rhs=t1[1],
                                             start=False, stop=False,
                                             skip_group_check=True)
                            nc.tensor.matmul(out=cs[:, :w], lhsT=mk,
                                             rhs=p_sb[:, 2 * h + 1, c0:c0 + w],
                                             start=False, stop=True)
                    nc.scalar.copy(out=compat_sb[0:20, c0:c0 + w], in_=cs[:, :w])

                # gather pickup (group 0) / delivery (group 1) to node order
                pd_g = spool.tile([128, NIDX], F32)
                nc.gpsimd.indirect_copy(pd_g[:], compat_sb[:], pdidx_sb[:],
                                        i_know_ap_gather_is_preferred=True)
                # stack: rows 0-3 pickup, rows 4-7 <- rows 16-19, rows 8-11 sig
                nc.sync.dma_start(out=pd_g[4:8, :], in_=pd_g[16:20, :])
                nc.sync.dma_start(out=pd_g[8:12, 0:N], in_=sig_d[b])

                # MLP: x1 = relu(W1 @ feats + b1eff) as a single 12-row stream
                # (rhs = high-half bf16 view of the f32 stack)
                pd_bf = pd_g[0:12, :].bitcast(BF16)
                x1_sb = spool.tile([32, NIDX], F16)
                x2_sb = spool.tile([32, NIDX], F16)
                tab_sb = spool.tile([1, NIDX], F32)
                for (c0, w) in MLP_CHUNKS:
                    x1p = mlp_ps.tile([32, 504], F32, space="PSUM", tag="m")
                    nc.tensor.matmul(out=x1p[:, :w], lhsT=w1t_sb[:],
                                     rhs=pd_bf[:, 2 * c0 + 1:2 * (c0 + w):2],
                                     start=True, stop=True)
                    nc.scalar.activation(out=x1_sb[:, c0:c0 + w], in_=x1p[:, :w],
                                         func=mybir.ActivationFunctionType.Relu,
                                         bias=b1e_sb[:, b:b + 1], scale=1.0)
                for (c0, w) in MLP_CHUNKS:
                    x2p = mlp_ps.tile([32, 504], F32, space="PSUM", tag="m")
                    nc.tensor.matmul(out=x2p[:, :w], lhsT=w2t_sb[:], rhs=x1_sb[:, c0:c0 + w],
                                     start=True, stop=True)
                    nc.scalar.activation(out=x2_sb[:, c0:c0 + w], in_=x2p[:, :w],
                                         func=mybir.ActivationFunctionType.Relu,
                                         bias=b2_sb[:], scale=1.0)
                for (c0, w) in MLP_CHUNKS:
                    tp3 = mlp_ps.tile([1, 504], F32, space="PSUM", tag="m")
                    nc.tensor.matmul(out=tp3[:, :w], lhsT=w3t_sb[:], rhs=x2_sb[:, c0:c0 + w],
                                     start=True, stop=True)
                    nc.scalar.activation(out=tab_sb[:, c0:c0 + w], in_=tp3[:, :w],
                                         func=mybir.ActivationFunctionType.Tanh,
                                         bias=b3_sb[:], scale=1.0)

                # softmax over 6*tanh; values bounded in [-6, 6] so no max-shift needed
                ex_sb = spool.tile([1, N], F32)
                ssum = spool.tile([1, 1], F32)
                nc.scalar.activation(out=ex_sb[:], in_=tab_sb[:, 0:N],
                                     func=mybir.ActivationFunctionType.Exp,
                                     bias=0.0, scale=6.0, accum_out=ssum[:])
                rcp = spool.tile([1, 1], F32)
                nc.vector.reciprocal(rcp[:], ssum[:])
                probs = spool.tile([1, N], F32)
                nc.vector.tensor_scalar_mul(probs[:], ex_sb[:], rcp[:])
                nc.sync.dma_start(out=out_d[b:b + 1, :], in_=probs[:])
    nc.compile()
    return nc


def _decompose(perm):
    visited = np.zeros(GS, bool)
    order = []
    real = []
    for start in range(GS):
        if visited[start]:
            continue
        cyc = [start]
        visited[start] = True
        nxt = int(perm[start])
        while nxt != start:
            cyc.append(nxt)
            visited[nxt] = True
            nxt = int(perm[nxt])
        L = len(cyc)
        order.extend([cyc[-1]] + cyc + [cyc[0 % L], cyc[1 % L]])
        real.extend([False] + [True] * L + [False] * 2)
    assert len(order) <= PEXT, f"too many cycles: ext len {len(order)}"
    pad = PEXT - len(order)
    order.extend([0] * pad)
    real.extend([False] * pad)
    return np.asarray(order, np.int64), np.asarray(real, bool)


def _ext_len(perm):
    visited = np.zeros(GS, bool)
    ncyc = 0
    for start in range(GS):
        if not visited[start]:
            ncyc += 1
            visited[start] = True
            nxt = int(perm[start])
            while nxt != start:
                visited[nxt] = True
                nxt = int(perm[nxt])
    return GS + 3 * ncyc


def _idx_rows(pos):
    """Wrap a flat index list into [16, IDXW] for one gpsimd core group."""
    idx = np.zeros(NIDX, np.uint16)
    idx[:pos.shape[0]] = pos
    return idx.reshape(IDXW, 16).T.copy()


def _host_prep(inputs):
    h_em = np.asarray(inputs["h_em"], np.float32)
    rec = np.asarray(inputs["rec"], np.int64)
    sig = np.ascontiguousarray(np.asarray(inputs["selection_sig"], np.float32))
    Wn = np.asarray(inputs["W_node"], np.float64)
    Wg = np.asarray(inputs["W_graph"], np.float64)
    WQ = np.asarray(inputs["W_Q"], np.float64)
    WK = np.asarray(inputs["W_K"], np.float64)
    w1 = np.asarray(inputs["agg_w1"], np.float64)
    b1 = np.asarray(inputs["agg_b1"], np.float64)
    w2 = np.asarray(inputs["agg_w2"], np.float32)
    b2 = np.asarray(inputs["agg_b2"], np.float32)
    w3 = np.asarray(inputs["agg_w3"], np.float32)
    b3 = np.asarray(inputs["agg_b3"], np.float32)

    Mt = np.zeros((NH, D, D), np.float64)
    C = np.zeros((NH, D, D), np.float64)
    S = np.zeros((NH, D, D), np.float64)
    for h in range(NH):
        M = WQ[h] @ WK[h].T
        Mt[h] = Wn.T @ M @ Wn
        C[h] = Wn.T @ (M + M.T) @ Wg
        S[h] = Wg.T @ M @ Wg
    mt = np.concatenate([Mt[h].astype(np.float16) for h in range(NH)], axis=1)

    g = h_em.max(axis=1).astype(np.float64)                      # (BS, D)
    cvec = np.einsum("hdf,bf->bdh", C, g).astype(np.float32)     # (BS, D, NH)
    svec = np.einsum("bd,hdf,bf->bh", g, S, g)                   # (BS, NH)
    b1_eff = (b1[None, :] + svec @ (w1[:, 0:4] + w1[:, 4:8]).T).astype(np.float32)

    w1f = w1.astype(np.float32)
    # mask20: head h -> output partitions h and 16+h (all-ones columns)
    hmask = np.zeros((128, NH * 20), np.float16)
    for h in range(NH):
        hmask[:, h * 20 + h] = 1.0
        hmask[:, h * 20 + 16 + h] = 1.0
    w1t = np.concatenate([w1f[:, 0:4].T, w1f[:, 4:8].T, w1f[:, 8:12].T],
                         axis=0)                                  # [12, 32]
    shared = {
        "mt": mt,
        "hmask": hmask,
        "w1t": np.ascontiguousarray(w1t.astype(np.float32)),  # cast below
        "b2": b2.reshape(32, 1),
        "w2t": np.ascontiguousarray(w2.T.astype(np.float16)),
        "w3t": np.ascontiguousarray(w3.T.astype(np.float16)),
        "b3": b3.reshape(1, 1),
    }
    try:
        import ml_dtypes
        shared["w1t"] = np.ascontiguousarray(w1t.astype(ml_dtypes.bfloat16))
    except ImportError:
        shared["w1t"] = np.ascontiguousarray(
            ((w1t.astype(np.float32).view(np.uint32) >> 16).astype(np.uint16)))

    in_maps = []
    for core in range(NCORES):
        b0 = core * BPC
        hemt = np.empty((BPC, 128, PEXT), np.float16)
        fmt = np.empty((BPC, 128, PEXT), np.float16)
        pdidx = np.zeros((BPC, 128, IDXW), np.uint16)
        for bl in range(BPC):
            order, real = _decompose(rec[b0 + bl])
            et = h_em[b0 + bl][order].T                           # (128, PEXT) f32
            hemt[bl] = et.astype(np.float16)
            fm = np.zeros_like(et)
            fm[:, 0:PEXT - 3] = et[:, 1:PEXT - 2] - et[:, 3:PEXT]
            fmt[bl] = fm.astype(np.float16)
            pon = np.zeros(GS, np.int64)
            pon[order[real]] = np.nonzero(real)[0]
            pdidx[bl, 0:16] = _idx_rows(pon[1:N + 1])
            pdidx[bl, 16:32] = _idx_rows(pon[N + 1:2 * N + 1])
        m = {
            "hemt": hemt,
            "fmt": fmt,
            "sig": sig[b0:b0 + BPC],
            "pdidx": pdidx,
            "cvec": cvec[b0:b0 + BPC],
            "b1e": np.ascontiguousarray(b1_eff[b0:b0 + BPC].T),
        }
        m.update(shared)
        in_maps.append(m)
    return in_maps


def kernel(**inputs) -> np.ndarray:
    global PEXT
    rec = np.asarray(inputs["rec"], np.int64)
    need = max(_ext_len(rec[b]) for b in range(rec.shape[0]))
    want = max(2048, -(-need // 512) * 512)
    if want != PEXT or "nc" not in _CACHE:
        PEXT = want
        _CACHE["nc"] = _build_nc()
    nc = _CACHE["nc"]
    in_maps = _host_prep(inputs)
    res = run_bass_kernel_spmd(nc, in_maps, list(range(NCORES)))
    return np.concatenate([res.results[i]["out"] for i in range(NCORES)], axis=0)


# revision 18
# speedup vs baseline: 1.7831x; 1.0123x over previous
"""Trainium2 Bass kernel for nn_MultiHeadDecoder (sparse neighbour compat + MLP + softmax).

Strategy (data-parallel over batch, 8 batches per core):
 - Host: decompose the `rec` permutation into cycles and lay nodes out in tour
   order (with per-cycle pad columns) so predecessor / succ^2 lookups become
   free-dim column shifts on-chip.  Ship per-batch:
     hemt  = E = h_em[order].T           fp16 [128, PEXT]
     fmt   = F' where F'[j] = E[j+1]-E[j+3]  fp16 [128, PEXT]
   plus gather indices for the pickup/delivery pairing (per-16-partition-group
   index lists for gpsimd indirect_copy).
 - Algebra folding (host, float64): per-head Q/K projections collapse to one
   bilinear form per head:
     compat[p] = (A_h[:,p-1]+c_h).F[:,p] + (A_h[:,p]+c_h).E[:,p+2]  (+s)
   A_h = Mt_h^T E;  c_h rides the PSUM drain as a per-partition bias; s folds
   into the first MLP bias.
 - Device per batch: 4 fp16 128x128 matmuls (A, PSUM f32), drains with fused
   +c on ACT/DVE to fp16, 8 fp16 elementwise products (DVE 2x / Pool),
   column-sum matmuls with a head-duplicating mask writing compat to
   partitions {0-3, 16-19}, one gpsimd indirect_copy gathering pickup rows
   (group 0) and delivery rows (group 1) in node order, a row-move DMA +
   sig DMA to build a single [12, NIDX] MLP input, then the 12->32->32->1
   MLP (fp16 weights, single-stream x1) + tanh + softmax.
"""
import os
import sys
from contextlib import ExitStack

import numpy as np

for _p in ("/opt/trn_rl_repo", "/root/.axon_site/_ro/trn_rl_repo"):
    if os.path.isdir(_p) and _p not in sys.path:
        sys.path.insert(0, _p)

import concourse.bacc as bacc
import concourse.bass as bass
import concourse.mybir as mybir
import concourse.tile as tile
from concourse.bass_utils import run_bass_kernel_spmd
from concourse.library_config import mlp as _mlp_lib

F32 = mybir.dt.float32
F16 = mybir.dt.float16
BF16 = mybir.dt.bfloat16
BS, GS, D, NH = 64, 2001, 128, 4
N = GS // 2                 # 1000
NCORES = 8
BPC = BS // NCORES          # 8 batches per core
PEXT = 2048                 # extended tour positions (3 pads/cycle; grown if needed)
NIDX = 1008                 # padded gather count (>= N, %16 == 0)
IDXW = NIDX // 16           # 63
MLP_CHUNKS = [(0, 504), (504, 504)]

# engine assignment tuning knobs
N_DRAIN_DVE = int(os.environ.get("K_DRAIN_DVE", "4"))   # of 8 A-drains on DVE (rest ACT)
N_PROD_POOL = int(os.environ.get("K_PROD_POOL", "0"))   # of 4 product pairs on Pool (rest DVE)
EPOOL_BUFS = int(os.environ.get("K_EPOOL_BUFS", "2"))
PPOOL_BUFS = int(os.environ.get("K_PPOOL_BUFS", "2"))
PRI_A = int(os.environ.get("K_PRI_A", "0"))      # priority lift for A-phase insts
PRI_P = int(os.environ.get("K_PRI_P", "0"))      # priority lift for product insts

_CACHE = {}


def _chunks():
    out = []
    c0 = 0
    while c0 < PEXT:
        out.append((c0, min(512, PEXT - c0)))
        c0 += 512
    return out


def _build_nc():
    CHUNKS = _chunks()
    nc = bacc.Bacc(None, target_bir_lowering=False, debug=False)
    hemt_d = nc.dram_tensor("hemt", [BPC, 128, PEXT], F16, kind="ExternalInput")
    fmt_d = nc.dram_tensor("fmt", [BPC, 128, 2, PEXT], F16, kind="ExternalInput")
    sig_d = nc.dram_tensor("sig", [BPC, 4, N], F32, kind="ExternalInput")
    pdidx_d = nc.dram_tensor("pdidx", [BPC, 128, IDXW], mybir.dt.uint16, kind="ExternalInput")
    mt_d = nc.dram_tensor("mt", [128, NH * 128], F16, kind="ExternalInput")
    cvec_d = nc.dram_tensor("cvec", [BPC, 128, NH], F32, kind="ExternalInput")
    w1t_d = nc.dram_tensor("w1t", [12, 32], BF16, kind="ExternalInput")
    b1e_d = nc.dram_tensor("b1e", [32, BPC], F32, kind="ExternalInput")
    w2t_d = nc.dram_tensor("w2t", [32, 32], F16, kind="ExternalInput")
    b2_d = nc.dram_tensor("b2", [32, 1], F32, kind="ExternalInput")
    w3t_d = nc.dram_tensor("w3t", [32, 1], F16, kind="ExternalInput")
    b3_d = nc.dram_tensor("b3", [1, 1], F32, kind="ExternalInput")
    hmask_d = nc.dram_tensor("hmask", [128, NH * 20], F16, kind="ExternalInput")
    out_d = nc.dram_tensor("out", [BPC, N], F32, kind="ExternalOutput")

    with tile.TileContext(nc) as tc:
        with ExitStack() as ctx:
            const = ctx.enter_context(tc.tile_pool(name="const", bufs=1))
            inpool = ctx.enter_context(tc.tile_pool(name="inpool", bufs=3))
            epool = ctx.enter_context(
                tc.tile_pool(name="epool", bufs=EPOOL_BUFS if PEXT <= 2048 else 1))
            ppool = ctx.enter_context(tc.tile_pool(name="ppool", bufs=PPOOL_BUFS))
            spool = ctx.enter_context(tc.tile_pool(name="spool", bufs=3))
            a_ps = ctx.enter_context(tc.tile_pool(name="a_ps", bufs=2, space="PSUM"))
            cs_ps = ctx.enter_context(tc.tile_pool(name="cs_ps", bufs=2, space="PSUM"))
            mlp_ps = ctx.enter_context(tc.tile_pool(name="mlp_ps", bufs=2, space="PSUM"))

            nc.gpsimd.load_library(_mlp_lib)

            hmask_sb = const.tile([128, NH * 20], F16)
            nc.sync.dma_start(out=hmask_sb[:], in_=hmask_d[:])
            mt_sb = const.tile([128, NH * 128], F16)
            nc.sync.dma_start(out=mt_sb[:], in_=mt_d[:])
            w1t_sb = const.tile([12, 32], BF16)
            nc.sync.dma_start(out=w1t_sb[:], in_=w1t_d[:])
            b1e_sb = const.tile([32, BPC], F32)
            nc.sync.dma_start(out=b1e_sb[:], in_=b1e_d[:])
            w2t_sb = const.tile([32, 32], F16)
            nc.sync.dma_start(out=w2t_sb[:], in_=w2t_d[:])
            b2_sb = const.tile([32, 1], F32)
            nc.sync.dma_start(out=b2_sb[:], in_=b2_d[:])
            w3t_sb = const.tile([32, 1], F16)
            nc.sync.dma_start(out=w3t_sb[:], in_=w3t_d[:])
            b3_sb = const.tile([1, 1], F32)
            nc.sync.dma_start(out=b3_sb[:], in_=b3_d[:])
            # compat gather sources: [128, PEXT] f32, rows 0-3 & 16-19 live
            compat2 = []
            for i in range(2):
                t = const.tile([128, PEXT], F32, name=f"compat{i}")
                nc.gpsimd.memset(t[:], 0.0)
                compat2.append(t)

            for b in range(BPC):
                pdidx_sb = inpool.tile([128, IDXW], mybir.dt.uint16)
                nc.sync.dma_start(out=pdidx_sb[:], in_=pdidx_d[b])
                cv_sb = inpool.tile([128, NH], F32)
                nc.sync.dma_start(out=cv_sb[:], in_=cvec_d[b])

                with tc.high_priority(offset=PRI_A if b > 0 else 0):
                    e_sb = epool.tile([128, PEXT], F16)
                    nc.sync.dma_start(out=e_sb[:], in_=hemt_d[b])
                    f_sb = epool.tile([128, 2, PEXT], F16)
                    nc.sync.dma_start(out=f_sb[:], in_=fmt_d[b])

                    # A'_h = Mt_h^T E + c_h, drained PSUM->SBUF as fp16 (ACT/DVE)
                    a_bf = epool.tile([128, NH, PEXT], F16)
                    drain_i = 0
                    for h in range(NH):
                        cv = cv_sb[:, h:h + 1]
                        for pair0 in range(0, PEXT, 1024):
                            pw = min(1024, PEXT - pair0)
                            ap = a_ps.tile([128, 1024], F32, space="PSUM", tag="a")
                            for (c0, w) in [(c, min(512, pw - (c - pair0)))
                                            for c in range(pair0, pair0 + pw, 512)]:
                                nc.tensor.matmul(out=ap[:, c0 - pair0:c0 - pair0 + w],
                                                 lhsT=mt_sb[:, h * 128:(h + 1) * 128],
                                                 rhs=e_sb[:, c0:c0 + w], start=True, stop=True)
                            if drain_i % 8 < N_DRAIN_DVE:
                                nc.vector.tensor_scalar_add(a_bf[:, h, pair0:pair0 + pw],
                                                            ap[:, :pw], cv)
                            else:
                                nc.scalar.add(out=a_bf[:, h, pair0:pair0 + pw],
                                              in_=ap[:, :pw], add=cv)
                            drain_i += 1

                # products (fp16, 2x): p_sb[:, 2h, j] = A'_h[:, j] * F'[:, j]
                #   (F'[j] = F[j+1], stored shifted for even alignment)
                #                      p_sb[:, 2h+1, j] = A'_h[:, j] * E[:, j+2]
                # chunked in halves so colsum chunks unlock earlier
                # one op per head: in0 = A'_h broadcast over the 2-term dim,
                # in1 = [F'; E2shift] (host-combined), out = both product rows
                p_sb = ppool.tile([128, 2 * NH, PEXT], F16)
                for h in range(NH):
                    eng = nc.gpsimd if h < N_PROD_POOL else nc.vector
                    eng.tensor_mul(p_sb[:, 2 * h:2 * h + 2, :],
                                   a_bf[:, h:h + 1, :].broadcast_to([128, 2, PEXT]),
                                   f_sb[:])

                # compat[h, pos p] = colsum(P2_h)[p] + colsum(P1_h)[p-1],
                # written to partitions h and 16+h (mask20 duplicates heads)
                compat_sb = compat2[b % 2]
                for (c0, w) in CHUNKS:
                    cs = cs_ps.tile([20, 512], F32, space="PSUM", tag="cs")
                    for h in range(NH):
                        mk = hmask_sb[:, h * 20:(h + 1) * 20]
                        if h < NH - 1:
                            nc.tensor.matmul(out=cs[:, :w], lhsT=mk,
                                             rhs=p_sb[:, 2 * h + 1, c0:c0 + w],
                                             start=(h == 0), stop=False)
                            t1 = (cs[:, 1:w], p_sb[:, 2 * h, 0:w - 1]) if c0 == 0 else \
                                 (cs[:, :w], p_sb[:, 2 * h, c0 - 1:c0 - 1 + w])
                            nc.tensor.matmul(out=t1[0], lhsT=mk, rhs=t1[1],
                                             start=False, stop=False,
                                             skip_group_check=True)
                        else:
                            t1 = (cs[:, 1:w], p_sb[:, 2 * h, 0:w - 1]) if c0 == 0 else \
                                 (cs[:, :w], p_sb[:, 2 * h, c0 - 1:c0 - 1 + w])
                            nc.tensor.matmul(out=t1[0], lhsT=mk, rhs=t1[1],
                                             start=False, stop=False,
                                             skip_group_check=True)
                            nc.tensor.matmul(out=cs[:, :w], lhsT=mk,
                                             rhs=p_sb[:, 2 * h + 1, c0:c0 + w],
                                             start=False, stop=True)
                    nc.scalar.copy(out=compat_sb[0:20, c0:c0 + w], in_=cs[:, :w])

                # gather pickup (group 0) / delivery (group 1) to node order
                pd_g = spool.tile([128, NIDX], F32)
                nc.gpsimd.indirect_copy(pd_g[:], compat_sb[:], pdidx_sb[:],
                                        i_know_ap_gather_is_preferred=True)
                # stack: rows 0-3 pickup, rows 4-7 <- rows 16-19, rows 8-11 sig
                nc.sync.dma_start(out=pd_g[4:8, :], in_=pd_g[16:20, :])
                nc.sync.dma_start(out=pd_g[8:12, 0:N], in_=sig_d[b])

                # MLP: x1 = relu(W1 @ feats + b1eff) as a single 12-row stream
                # (rhs = high-half bf16 view of the f32 stack)
                pd_bf = pd_g[0:12, :].bitcast(BF16)
                x1_sb = spool.tile([32, NIDX], F16)
                x2_sb = spool.tile([32, NIDX], F16)
                tab_sb = spool.tile([1, NIDX], F32)
                for (c0, w) in MLP_CHUNKS:
                    x1p = mlp_ps.tile([32, 504], F32, space="PSUM", tag="m")
                    nc.tensor.matmul(out=x1p[:, :w], lhsT=w1t_sb[:],
                                     rhs=pd_bf[:, 2 * c0 + 1:2 * (c0 + w):2],
                                     start=True, stop=True)
                    nc.scalar.activation(out=x1_sb[:, c0:c0 + w], in_=x1p[:, :w],
                                         func=mybir.ActivationFunctionType.Relu,
                                         bias=b1e_sb[:, b:b + 1], scale=1.0)
                for (c0, w) in MLP_CHUNKS:
                    x2p = mlp_ps.tile([32, 504], F32, space="PSUM", tag="m")
                    nc.tensor.matmul(out=x2p[:, :w], lhsT=w2t_sb[:], rhs=x1_sb[:, c0:c0 + w],
                                     start=True, stop=True)
                    nc.scalar.activation(out=x2_sb[:, c0:c0 + w], in_=x2p[:, :w],
                                         func=mybir.ActivationFunctionType.Relu,
                                         bias=b2_sb[:], scale=1.0)
                for (c0, w) in MLP_CHUNKS:
                    tp3 = mlp_ps.tile([1, 504], F32, space="PSUM", tag="m")
                    nc.tensor.matmul(out=tp3[:, :w], lhsT=w3t_sb[:], rhs=x2_sb[:, c0:c0 + w],
                                     start=True, stop=True)
                    nc.scalar.activation(out=tab_sb[:, c0:c0 + w], in_=tp3[:, :w],
                                         func=mybir.ActivationFunctionType.Tanh,
                                         bias=b3_sb[:], scale=1.0)

                # softmax over 6*tanh; values bounded in [-6, 6] so no max-shift needed
                ex_sb = spool.tile([1, N], F32)
                ssum = spool.tile([1, 1], F32)
                nc.scalar.activation(out=ex_sb[:], in_=tab_sb[:, 0:N],
                                     func=mybir.ActivationFunctionType.Exp,
                                     bias=0.0, scale=6.0, accum_out=ssum[:])
                rcp = spool.tile([1, 1], F32)
                nc.vector.reciprocal(rcp[:], ssum[:])
                probs = spool.tile([1, N], F32)
                nc.vector.tensor_scalar_mul(probs[:], ex_sb[:], rcp[:])
                nc.sync.dma_start(out=out_d[b:b + 1, :], in_=probs[:])
    nc.compile()
    return nc


def _decompose(perm):
    visited = np.zeros(GS, bool)
    order = []
    real = []
    for start in range(GS):
        if visited[start]:
            continue
        cyc = [start]
        visited[start] = True
        nxt = int(perm[start])
        while nxt != start:
            cyc.append(nxt)
            visited[nxt] = True
            nxt = int(perm[nxt])
        L = len(cyc)
        order.extend([cyc[-1]] + cyc + [cyc[0 % L], cyc[1 % L]])
        real.extend([False] + [True] * L + [False] * 2)
    assert len(order) <= PEXT, f"too many cycles: ext len {len(order)}"
    pad = PEXT - len(order)
    order.extend([0] * pad)
    real.extend([False] * pad)
    return np.asarray(order, np.int64), np.asarray(real, bool)


def _ext_len(perm):
    visited = np.zeros(GS, bool)
    ncyc = 0
    for start in range(GS):
        if not visited[start]:
            ncyc += 1
            visited[start] = True
            nxt = int(perm[start])
            while nxt != start:
                visited[nxt] = True
                nxt = int(perm[nxt])
    return GS + 3 * ncyc


def _idx_rows(pos):
    """Wrap a flat index list into [16, IDXW] for one gpsimd core group."""
    idx = np.zeros(NIDX, np.uint16)
    idx[:pos.shape[0]] = pos
    return idx.reshape(IDXW, 16).T.copy()


def _host_prep(inputs):
    h_em = np.asarray(inputs["h_em"], np.float32)
    rec = np.asarray(inputs["rec"], np.int64)
    sig = np.ascontiguousarray(np.asarray(inputs["selection_sig"], np.float32))
    Wn = np.asarray(inputs["W_node"], np.float64)
    Wg = np.asarray(inputs["W_graph"], np.float64)
    WQ = np.asarray(inputs["W_Q"], np.float64)
    WK = np.asarray(inputs["W_K"], np.float64)
    w1 = np.asarray(inputs["agg_w1"], np.float64)
    b1 = np.asarray(inputs["agg_b1"], np.float64)
    w2 = np.asarray(inputs["agg_w2"], np.float32)
    b2 = np.asarray(inputs["agg_b2"], np.float32)
    w3 = np.asarray(inputs["agg_w3"], np.float32)
    b3 = np.asarray(inputs["agg_b3"], np.float32)

    Mt = np.zeros((NH, D, D), np.float64)
    C = np.zeros((NH, D, D), np.float64)
    S = np.zeros((NH, D, D), np.float64)
    for h in range(NH):
        M = WQ[h] @ WK[h].T
        Mt[h] = Wn.T @ M @ Wn
        C[h] = Wn.T @ (M + M.T) @ Wg
        S[h] = Wg.T @ M @ Wg
    mt = np.concatenate([Mt[h].astype(np.float16) for h in range(NH)], axis=1)

    g = h_em.max(axis=1).astype(np.float64)                      # (BS, D)
    cvec = np.einsum("hdf,bf->bdh", C, g).astype(np.float32)     # (BS, D, NH)
    svec = np.einsum("bd,hdf,bf->bh", g, S, g)                   # (BS, NH)
    b1_eff = (b1[None, :] + svec @ (w1[:, 0:4] + w1[:, 4:8]).T).astype(np.float32)

    w1f = w1.astype(np.float32)
    # mask20: head h -> output partitions h and 16+h (all-ones columns)
    hmask = np.zeros((128, NH * 20), np.float16)
    for h in range(NH):
        hmask[:, h * 20 + h] = 1.0
        hmask[:, h * 20 + 16 + h] = 1.0
    w1t = np.concatenate([w1f[:, 0:4].T, w1f[:, 4:8].T, w1f[:, 8:12].T],
                         axis=0)                                  # [12, 32]
    shared = {
        "mt": mt,
        "hmask": hmask,
        "w1t": np.ascontiguousarray(w1t.astype(np.float32)),  # cast below
        "b2": b2.reshape(32, 1),
        "w2t": np.ascontiguousarray(w2.T.astype(np.float16)),
        "w3t": np.ascontiguousarray(w3.T.astype(np.float16)),
        "b3": b3.reshape(1, 1),
    }
    try:
        import ml_dtypes
        shared["w1t"] = np.ascontiguousarray(w1t.astype(ml_dtypes.bfloat16))
    except ImportError:
        shared["w1t"] = np.ascontiguousarray(
            ((w1t.astype(np.float32).view(np.uint32) >> 16).astype(np.uint16)))

    in_maps = []
    for core in range(NCORES):
        b0 = core * BPC
        hemt = np.empty((BPC, 128, PEXT), np.float16)
        fmt = np.empty((BPC, 128, 2, PEXT), np.float16)
        pdidx = np.zeros((BPC, 128, IDXW), np.uint16)
        for bl in range(BPC):
            order, real = _decompose(rec[b0 + bl])
            et = h_em[b0 + bl][order].T                           # (128, PEXT) f32
            hemt[bl] = et.astype(np.float16)
            fm = np.zeros_like(et)
            fm[:, 0:PEXT - 3] = et[:, 1:PEXT - 2] - et[:, 3:PEXT]
            fmt[bl, :, 0, :] = fm.astype(np.float16)
            e2 = np.concatenate([et[:, 2:PEXT], et[:, PEXT - 2:PEXT]], axis=1)
            fmt[bl, :, 1, :] = e2.astype(np.float16)
            pon = np.zeros(GS, np.int64)
            pon[order[real]] = np.nonzero(real)[0]
            pdidx[bl, 0:16] = _idx_rows(pon[1:N + 1])
            pdidx[bl, 16:32] = _idx_rows(pon[N + 1:2 * N + 1])
        m = {
            "hemt": hemt,
            "fmt": fmt,
            "sig": sig[b0:b0 + BPC],
            "pdidx": pdidx,
            "cvec": cvec[b0:b0 + BPC],
            "b1e": np.ascontiguousarray(b1_eff[b0:b0 + BPC].T),
        }
        m.update(shared)
        in_maps.append(m)
    return in_maps


def kernel(**inputs) -> np.ndarray:
    global PEXT
    rec = np.asarray(inputs["rec"], np.int64)
    need = max(_ext_len(rec[b]) for b in range(rec.shape[0]))
    want = max(1024, -(-need // 16) * 16)
    if want != PEXT or "nc" not in _CACHE:
        PEXT = want
        _CACHE["nc"] = _build_nc()
    nc = _CACHE["nc"]
    in_maps = _host_prep(inputs)
    res = run_bass_kernel_spmd(nc, in_maps, list(range(NCORES)))
    return np.concatenate([res.results[i]["out"] for i in range(NCORES)], axis=0)


# revision 21
# speedup vs baseline: 1.9959x; 1.1193x over previous
"""Trainium2 Bass kernel for nn_MultiHeadDecoder (sparse neighbour compat + MLP + softmax).

Strategy (data-parallel over batch, 8 batches per core):
 - Host: decompose the `rec` permutation into cycles and lay nodes out in tour
   order (with per-cycle pad columns) so predecessor / succ^2 lookups become
   free-dim column shifts on-chip.  Ship per-batch:
     hemt  = E = h_em[order].T           fp16 [128, PEXT]
     fmt   = F' where F'[j] = E[j+1]-E[j+3]  fp16 [128, PEXT]
   plus gather indices for the pickup/delivery pairing (per-16-partition-group
   index lists for gpsimd indirect_copy).
 - Algebra folding (host, float64): per-head Q/K projections collapse to one
   bilinear form per head:
     compat[p] = (A_h[:,p-1]+c_h).F[:,p] + (A_h[:,p]+c_h).E[:,p+2]  (+s)
   A_h = Mt_h^T E;  c_h rides the PSUM drain as a per-partition bias; s folds
   into the first MLP bias.
 - Device per batch: 4 fp16 128x128 matmuls (A, PSUM f32), drains with fused
   +c on ACT/DVE to fp16, 8 fp16 elementwise products (DVE 2x / Pool),
   column-sum matmuls with a head-duplicating mask writing compat to
   partitions {0-3, 16-19}, one gpsimd indirect_copy gathering pickup rows
   (group 0) and delivery rows (group 1) in node order, a row-move DMA +
   sig DMA to build a single [12, NIDX] MLP input, then the 12->32->32->1
   MLP (fp16 weights, single-stream x1) + tanh + softmax.
"""
import os
import sys
from contextlib import ExitStack

import numpy as np

for _p in ("/opt/trn_rl_repo", "/root/.axon_site/_ro/trn_rl_repo"):
    if os.path.isdir(_p) and _p not in sys.path:
        sys.path.insert(0, _p)

import concourse.bacc as bacc
import concourse.bass as bass
import concourse.mybir as mybir
import concourse.tile as tile
from concourse.bass_utils import run_bass_kernel_spmd
from concourse.library_config import mlp as _mlp_lib

F32 = mybir.dt.float32
F16 = mybir.dt.float16
BF16 = mybir.dt.bfloat16
BS, GS, D, NH = 64, 2001, 128, 4
N = GS // 2                 # 1000
NCORES = 8
BPC = BS // NCORES          # 8 batches per core
PEXT = 2048                 # extended tour positions (3 pads/cycle; grown if needed)
NIDX = 1008                 # padded gather count (>= N, %16 == 0)
IDXW = NIDX // 16           # 63
MLP_CHUNKS = [(0, 504), (504, 504)]

# engine assignment tuning knobs
N_DRAIN_DVE = int(os.environ.get("K_DRAIN_DVE", "3"))   # of 8 A-drains on DVE (rest ACT)
N_PROD_POOL = int(os.environ.get("K_PROD_POOL", "0"))   # of 4 product pairs on Pool (rest DVE)
EPOOL_BUFS = int(os.environ.get("K_EPOOL_BUFS", "2"))
PPOOL_BUFS = int(os.environ.get("K_PPOOL_BUFS", "2"))
PRI_A = int(os.environ.get("K_PRI_A", "95"))      # priority lift for A-phase insts
PRI_P = int(os.environ.get("K_PRI_P", "0"))      # priority lift for product insts

_CACHE = {}


def _chunks():
    out = []
    c0 = 0
    while c0 < PEXT:
        out.append((c0, min(512, PEXT - c0)))
        c0 += 512
    return out


def _build_nc():
    CHUNKS = _chunks()
    nc = bacc.Bacc(None, target_bir_lowering=False, debug=False)
    hemt_d = nc.dram_tensor("hemt", [BPC, 128, PEXT], F16, kind="ExternalInput")
    fmt_d = nc.dram_tensor("fmt", [BPC, 128, 2, PEXT], F16, kind="ExternalInput")
    sig_d = nc.dram_tensor("sig", [BPC, 4, N], F32, kind="ExternalInput")
    pdidx_d = nc.dram_tensor("pdidx", [BPC, 128, IDXW], mybir.dt.uint16, kind="ExternalInput")
    mt_d = nc.dram_tensor("mt", [128, NH * 128], F16, kind="ExternalInput")
    cvec_d = nc.dram_tensor("cvec", [BPC, 128, NH], F32, kind="ExternalInput")
    w1t_d = nc.dram_tensor("w1t", [12, 32], BF16, kind="ExternalInput")
    b1e_d = nc.dram_tensor("b1e", [32, BPC], F32, kind="ExternalInput")
    w2t_d = nc.dram_tensor("w2t", [32, 32], F16, kind="ExternalInput")
    b2_d = nc.dram_tensor("b2", [32, 1], F32, kind="ExternalInput")
    w3t_d = nc.dram_tensor("w3t", [32, 1], F16, kind="ExternalInput")
    b3_d = nc.dram_tensor("b3", [1, 1], F32, kind="ExternalInput")
    hmask_d = nc.dram_tensor("hmask", [128, NH * 20], F16, kind="ExternalInput")
    out_d = nc.dram_tensor("out", [BPC, N], F32, kind="ExternalOutput")

    with tile.TileContext(nc) as tc:
        with ExitStack() as ctx:
            const = ctx.enter_context(tc.tile_pool(name="const", bufs=1))
            inpool = ctx.enter_context(tc.tile_pool(name="inpool", bufs=3))
            epool = ctx.enter_context(
                tc.tile_pool(name="epool", bufs=EPOOL_BUFS if PEXT <= 2048 else 1))
            ppool = ctx.enter_context(tc.tile_pool(name="ppool", bufs=PPOOL_BUFS))
            spool = ctx.enter_context(tc.tile_pool(name="spool", bufs=3))
            a_ps = ctx.enter_context(tc.tile_pool(name="a_ps", bufs=2, space="PSUM"))
            cs_ps = ctx.enter_context(tc.tile_pool(name="cs_ps", bufs=2, space="PSUM"))
            mlp_ps = ctx.enter_context(tc.tile_pool(name="mlp_ps", bufs=2, space="PSUM"))

            nc.gpsimd.load_library(_mlp_lib)

            hmask_sb = const.tile([128, NH * 20], F16)
            nc.sync.dma_start(out=hmask_sb[:], in_=hmask_d[:])
            mt_sb = const.tile([128, NH * 128], F16)
            nc.sync.dma_start(out=mt_sb[:], in_=mt_d[:])
            w1t_sb = const.tile([12, 32], BF16)
            nc.sync.dma_start(out=w1t_sb[:], in_=w1t_d[:])
            b1e_sb = const.tile([32, BPC], F32)
            nc.sync.dma_start(out=b1e_sb[:], in_=b1e_d[:])
            w2t_sb = const.tile([32, 32], F16)
            nc.sync.dma_start(out=w2t_sb[:], in_=w2t_d[:])
            b2_sb = const.tile([32, 1], F32)
            nc.sync.dma_start(out=b2_sb[:], in_=b2_d[:])
            w3t_sb = const.tile([32, 1], F16)
            nc.sync.dma_start(out=w3t_sb[:], in_=w3t_d[:])
            b3_sb = const.tile([1, 1], F32)
            nc.sync.dma_start(out=b3_sb[:], in_=b3_d[:])
            # compat gather sources: [128, PEXT] f32, rows 0-3 & 16-19 live
            compat2 = []
            for i in range(2):
                t = const.tile([128, PEXT], F32, name=f"compat{i}")
                nc.gpsimd.memset(t[:], 0.0)
                compat2.append(t)

            for b in range(BPC):
                pdidx_sb = inpool.tile([128, IDXW], mybir.dt.uint16)
                nc.sync.dma_start(out=pdidx_sb[:], in_=pdidx_d[b])
                cv_sb = inpool.tile([128, NH], F32)
                nc.sync.dma_start(out=cv_sb[:], in_=cvec_d[b])

                with tc.high_priority(offset=PRI_A if b > 0 else 0):
                    e_sb = epool.tile([128, PEXT], F16)
                    nc.sync.dma_start(out=e_sb[:], in_=hemt_d[b])
                    f_sb = epool.tile([128, 2, PEXT], F16)
                    nc.sync.dma_start(out=f_sb[:], in_=fmt_d[b])

                    # A'_h = Mt_h^T E + c_h, drained PSUM->SBUF as fp16 (ACT/DVE)
                    a_bf = epool.tile([128, NH, PEXT], F16)
                    drain_i = 0
                    for h in range(NH):
                        cv = cv_sb[:, h:h + 1]
                        for pair0 in range(0, PEXT, 1024):
                            pw = min(1024, PEXT - pair0)
                            ap = a_ps.tile([128, 1024], F32, space="PSUM", tag="a")
                            for (c0, w) in [(c, min(512, pw - (c - pair0)))
                                            for c in range(pair0, pair0 + pw, 512)]:
                                nc.tensor.matmul(out=ap[:, c0 - pair0:c0 - pair0 + w],
                                                 lhsT=mt_sb[:, h * 128:(h + 1) * 128],
                                                 rhs=e_sb[:, c0:c0 + w], start=True, stop=True)
                            if drain_i % 8 < N_DRAIN_DVE:
                                nc.vector.tensor_scalar_add(a_bf[:, h, pair0:pair0 + pw],
                                                            ap[:, :pw], cv)
                            else:
                                nc.scalar.add(out=a_bf[:, h, pair0:pair0 + pw],
                                              in_=ap[:, :pw], add=cv)
                            drain_i += 1

                # products (fp16, 2x): p_sb[:, 2h, j] = A'_h[:, j] * F'[:, j]
                #   (F'[j] = F[j+1], stored shifted for even alignment)
                #                      p_sb[:, 2h+1, j] = A'_h[:, j] * E[:, j+2]
                # chunked in halves so colsum chunks unlock earlier
                # one op per head: in0 = A'_h broadcast over the 2-term dim,
                # in1 = [F'; E2shift] (host-combined), out = both product rows
                p_sb = ppool.tile([128, 2 * NH, PEXT], F16)
                for h in range(NH):
                    if h < NH - N_PROD_POOL:
                        nc.vector.tensor_mul(p_sb[:, 2 * h:2 * h + 2, :],
                                             a_bf[:, h:h + 1, :].broadcast_to([128, 2, PEXT]),
                                             f_sb[:])
                    else:
                        # split this head: F-plane on Pool, E2-plane on DVE
                        nc.gpsimd.tensor_mul(p_sb[:, 2 * h, :], a_bf[:, h, :],
                                             f_sb[:, 0, :])
                        nc.vector.tensor_mul(p_sb[:, 2 * h + 1, :], a_bf[:, h, :],
                                             f_sb[:, 1, :])

                # compat[h, pos p] = colsum(P2_h)[p] + colsum(P1_h)[p-1],
                # written to partitions h and 16+h (mask20 duplicates heads)
                compat_sb = compat2[b % 2]
                for (c0, w) in CHUNKS:
                    cs = cs_ps.tile([20, 512], F32, space="PSUM", tag="cs")
                    for h in range(NH):
                        mk = hmask_sb[:, h * 20:(h + 1) * 20]
                        if h < NH - 1:
                            nc.tensor.matmul(out=cs[:, :w], lhsT=mk,
                                             rhs=p_sb[:, 2 * h + 1, c0:c0 + w],
                                             start=(h == 0), stop=False)
                            t1 = (cs[:, 1:w], p_sb[:, 2 * h, 0:w - 1]) if c0 == 0 else \
                                 (cs[:, :w], p_sb[:, 2 * h, c0 - 1:c0 - 1 + w])
                            nc.tensor.matmul(out=t1[0], lhsT=mk, rhs=t1[1],
                                             start=False, stop=False,
                                             skip_group_check=True)
                        else:
                            t1 = (cs[:, 1:w], p_sb[:, 2 * h, 0:w - 1]) if c0 == 0 else \
                                 (cs[:, :w], p_sb[:, 2 * h, c0 - 1:c0 - 1 + w])
                            nc.tensor.matmul(out=t1[0], lhsT=mk, rhs=t1[1],
                                             start=False, stop=False,
                                             skip_group_check=True)
                            nc.tensor.matmul(out=cs[:, :w], lhsT=mk,
                                             rhs=p_sb[:, 2 * h + 1, c0:c0 + w],
                                             start=False, stop=True)
                    nc.scalar.copy(out=compat_sb[0:20, c0:c0 + w], in_=cs[:, :w])

                # gather pickup (group 0) / delivery (group 1) to node order
                pd_g = spool.tile([128, NIDX], F32)
                nc.gpsimd.indirect_copy(pd_g[:], compat_sb[:], pdidx_sb[:],
                                        i_know_ap_gather_is_preferred=True)
                # stack: rows 0-3 pickup, rows 4-7 <- rows 16-19, rows 8-11 sig
                nc.sync.dma_start(out=pd_g[4:8, :], in_=pd_g[16:20, :])
                nc.sync.dma_start(out=pd_g[8:12, 0:N], in_=sig_d[b])

                # MLP: x1 = relu(W1 @ feats + b1eff) as a single 12-row stream
                # (rhs = high-half bf16 view of the f32 stack)
                pd_bf = pd_g[0:12, :].bitcast(BF16)
                x1_sb = spool.tile([32, NIDX], F16)
                x2_sb = spool.tile([32, NIDX], F16)
                tab_sb = spool.tile([1, NIDX], F32)
                for (c0, w) in MLP_CHUNKS:
                    x1p = mlp_ps.tile([32, 504], F32, space="PSUM", tag="m")
                    nc.tensor.matmul(out=x1p[:, :w], lhsT=w1t_sb[:],
                                     rhs=pd_bf[:, 2 * c0 + 1:2 * (c0 + w):2],
                                     start=True, stop=True)
                    nc.scalar.activation(out=x1_sb[:, c0:c0 + w], in_=x1p[:, :w],
                                         func=mybir.ActivationFunctionType.Relu,
                                         bias=b1e_sb[:, b:b + 1], scale=1.0)
                for (c0, w) in MLP_CHUNKS:
                    x2p = mlp_ps.tile([32, 504], F32, space="PSUM", tag="m")
                    nc.tensor.matmul(out=x2p[:, :w], lhsT=w2t_sb[:], rhs=x1_sb[:, c0:c0 + w],
                                     start=True, stop=True)
                    nc.scalar.activation(out=x2_sb[:, c0:c0 + w], in_=x2p[:, :w],
                                         func=mybir.ActivationFunctionType.Relu,
                                         bias=b2_sb[:], scale=1.0)
                for (c0, w) in MLP_CHUNKS:
                    tp3 = mlp_ps.tile([1, 504], F32, space="PSUM", tag="m")
                    nc.tensor.matmul(out=tp3[:, :w], lhsT=w3t_sb[:], rhs=x2_sb[:, c0:c0 + w],
                                     start=True, stop=True)
                    nc.scalar.activation(out=tab_sb[:, c0:c0 + w], in_=tp3[:, :w],
                                         func=mybir.ActivationFunctionType.Tanh,
                                         bias=b3_sb[:], scale=1.0)

                # softmax over 6*tanh; values bounded in [-6, 6] so no max-shift needed
                ex_sb = spool.tile([1, N], F32)
                ssum = spool.tile([1, 1], F32)
                nc.scalar.activation(out=ex_sb[:], in_=tab_sb[:, 0:N],
                                     func=mybir.ActivationFunctionType.Exp,
                                     bias=0.0, scale=6.0, accum_out=ssum[:])
                rcp = spool.tile([1, 1], F32)
                nc.vector.reciprocal(rcp[:], ssum[:])
                probs = spool.tile([1, N], F32)
                if os.environ.get("K_PROBS_POOL", "0") == "1":
                    nc.gpsimd.tensor_scalar_mul(probs[:], ex_sb[:], rcp[:])
                else:
                    nc.vector.tensor_scalar_mul(probs[:], ex_sb[:], rcp[:])
                nc.sync.dma_start(out=out_d[b:b + 1, :], in_=probs[:])
    nc.compile()
    return nc


def _decompose(perm):
    visited = np.zeros(GS, bool)
    order = []
    real = []
    for start in range(GS):
        if visited[start]:
            continue
        cyc = [start]
        visited[start] = True
        nxt = int(perm[start])
        while nxt != start:
            cyc.append(nxt)
            visited[nxt] = True
            nxt = int(perm[nxt])
        L = len(cyc)
        order.extend([cyc[-1]] + cyc + [cyc[0 % L], cyc[1 % L]])
        real.extend([False] + [True] * L + [False] * 2)
    assert len(order) <= PEXT, f"too many cycles: ext len {len(order)}"
    pad = PEXT - len(order)
    order.extend([0] * pad)
    real.extend([False] * pad)
    return np.asarray(order, np.int64), np.asarray(real, bool)


def _ext_len(perm):
    visited = np.zeros(GS, bool)
    ncyc = 0
    for start in range(GS):
        if not visited[start]:
            ncyc += 1
            visited[start] = True
            nxt = int(perm[start])
            while nxt != start:
                visited[nxt] = True
                nxt = int(perm[nxt])
    return GS + 3 * ncyc


def _idx_rows(pos):
    """Wrap a flat index list into [16, IDXW] for one gpsimd core group."""
    idx = np.zeros(NIDX, np.uint16)
    idx[:pos.shape[0]] = pos
    return idx.reshape(IDXW, 16).T.copy()


def _host_prep(inputs):
    h_em = np.asarray(inputs["h_em"], np.float32)
    rec = np.asarray(inputs["rec"], np.int64)
    sig = np.ascontiguousarray(np.asarray(inputs["selection_sig"], np.float32))
    Wn = np.asarray(inputs["W_node"], np.float64)
    Wg = np.asarray(inputs["W_graph"], np.float64)
    WQ = np.asarray(inputs["W_Q"], np.float64)
    WK = np.asarray(inputs["W_K"], np.float64)
    w1 = np.asarray(inputs["agg_w1"], np.float64)
    b1 = np.asarray(inputs["agg_b1"], np.float64)
    w2 = np.asarray(inputs["agg_w2"], np.float32)
    b2 = np.asarray(inputs["agg_b2"], np.float32)
    w3 = np.asarray(inputs["agg_w3"], np.float32)
    b3 = np.asarray(inputs["agg_b3"], np.float32)

    Mt = np.zeros((NH, D, D), np.float64)
    C = np.zeros((NH, D, D), np.float64)
    S = np.zeros((NH, D, D), np.float64)
    for h in range(NH):
        M = WQ[h] @ WK[h].T
        Mt[h] = Wn.T @ M @ Wn
        C[h] = Wn.T @ (M + M.T) @ Wg
        S[h] = Wg.T @ M @ Wg
    mt = np.concatenate([Mt[h].astype(np.float16) for h in range(NH)], axis=1)

    g = h_em.max(axis=1).astype(np.float64)                      # (BS, D)
    cvec = np.einsum("hdf,bf->bdh", C, g).astype(np.float32)     # (BS, D, NH)
    svec = np.einsum("bd,hdf,bf->bh", g, S, g)                   # (BS, NH)
    b1_eff = (b1[None, :] + svec @ (w1[:, 0:4] + w1[:, 4:8]).T).astype(np.float32)

    w1f = w1.astype(np.float32)
    # mask20: head h -> output partitions h and 16+h (all-ones columns)
    hmask = np.zeros((128, NH * 20), np.float16)
    for h in range(NH):
        hmask[:, h * 20 + h] = 1.0
        hmask[:, h * 20 + 16 + h] = 1.0
    w1t = np.concatenate([w1f[:, 0:4].T, w1f[:, 4:8].T, w1f[:, 8:12].T],
                         axis=0)                                  # [12, 32]
    shared = {
        "mt": mt,
        "hmask": hmask,
        "w1t": np.ascontiguousarray(w1t.astype(np.float32)),  # cast below
        "b2": b2.reshape(32, 1),
        "w2t": np.ascontiguousarray(w2.T.astype(np.float16)),
        "w3t": np.ascontiguousarray(w3.T.astype(np.float16)),
        "b3": b3.reshape(1, 1),
    }
    try:
        import ml_dtypes
        shared["w1t"] = np.ascontiguousarray(w1t.astype(ml_dtypes.bfloat16))
    except ImportError:
        shared["w1t"] = np.ascontiguousarray(
            ((w1t.astype(np.float32).view(np.uint32) >> 16).astype(np.uint16)))

    in_maps = []
    for core in range(NCORES):
        b0 = core * BPC
        hemt = np.empty((BPC, 128, PEXT), np.float16)
        fmt = np.empty((BPC, 128, 2, PEXT), np.float16)
        pdidx = np.zeros((BPC, 128, IDXW), np.uint16)
        for bl in range(BPC):
            order, real = _decompose(rec[b0 + bl])
            et = h_em[b0 + bl][order].T                           # (128, PEXT) f32
            hemt[bl] = et.astype(np.float16)
            fm = np.zeros_like(et)
            fm[:, 0:PEXT - 3] = et[:, 1:PEXT - 2] - et[:, 3:PEXT]
            fmt[bl, :, 0, :] = fm.astype(np.float16)
            e2 = np.concatenate([et[:, 2:PEXT], et[:, PEXT - 2:PEXT]], axis=1)
            fmt[bl, :, 1, :] = e2.astype(np.float16)
            pon = np.zeros(GS, np.int64)
            pon[order[real]] = np.nonzero(real)[0]
            pdidx[bl, 0:16] = _idx_rows(pon[1:N + 1])
            pdidx[bl, 16:32] = _idx_rows(pon[N + 1:2 * N + 1])
        m = {
            "hemt": hemt,
            "fmt": fmt,
            "sig": sig[b0:b0 + BPC],
            "pdidx": pdidx,
            "cvec": cvec[b0:b0 + BPC],
            "b1e": np.ascontiguousarray(b1_eff[b0:b0 + BPC].T),
        }
        m.update(shared)
        in_maps.append(m)
    return in_maps


def kernel(**inputs) -> np.ndarray:
    global PEXT
    rec = np.asarray(inputs["rec"], np.int64)
    need = max(_ext_len(rec[b]) for b in range(rec.shape[0]))
    want = max(1024, -(-need // 16) * 16)
    if want != PEXT or "nc" not in _CACHE:
        PEXT = want
        _CACHE["nc"] = _build_nc()
    nc = _CACHE["nc"]
    in_maps = _host_prep(inputs)
    res = run_bass_kernel_spmd(nc, in_maps, list(range(NCORES)))
    return np.concatenate([res.results[i]["out"] for i in range(NCORES)], axis=0)
